# revision 7
# baseline (speedup 1.0000x reference)
"""CondNet kernel for Trainium2 (8 NeuronCores, model-parallel over width).

Model (reference):
    h1 = relu(x @ W_in.T + b_in)                     # (512, 8192)
    h  = relu(condensed(h, W_mid[i], b_mid[i]))      # i in {0, 1}; gather + weighted sum
    out = h @ W_out.T + b_out                        # (512, 1000)

Strategy (v1, model-parallel):
  - The condensed (gather) layers are re-expressed as dense matmuls
    h @ S where S[k, o] = sum_f W_mid[o, f] * [indx_seqs[o, f] == k],
    built on the host (compile-time transform of the weights+indices).
  - Every layer's OUTPUT dim is sharded across the 8 cores (1024 mid
    units each); batch (512) is replicated.  Per-core weight traffic is
    ~36MB instead of ~288MB (data-parallel), flipping the kernel from
    HBM-bound to PE-bound.
  - PSUM layout is [o_partition, batch_free] with the weights as the
    stationary operand, so each layer's activation output is already
    k-major for the next layer: no transposes at all.
  - After layers 1 and 2 an AllGather (HBM, TOPSP/SDMA) rebuilds the full
    k-major activation [8192, 512] on every core.  The final layer is
    k-sharded (each core contracts its own h3 chunk against the matching
    W_out rows) and finishes with a ReduceScatter over the batch dim.
  - All weights/activations bf16, accumulation in fp32 PSUM.
  - Final bias b_out is added on the host (free).
"""

import os
import numpy as np
import ml_dtypes

import concourse.bass as bass
import concourse.tile as tile
from concourse import bacc, mybir
from concourse import bass_utils

BF16 = ml_dtypes.bfloat16

B, NUM_IN, NUM_MID, NUM_OUT, FAN_IN, N_COND = 512, 1024, 8192, 1000, 64, 2
NCORES = 8
OC = NUM_MID // NCORES          # 1024 output (mid) units per core
BC = B // NCORES                # 64 batch rows per core in the final output
OPAD = 1024                     # padded out-layer width
NKT = NUM_MID // 128            # 64 k-tiles in a condensed contraction
RG = [list(range(NCORES))]      # one replica group: all 8 cores

_cache = {}
LAST_RESULT = None  # BassKernelResults of the most recent run (for test harness)


def _build_nc():
    """Build + compile the Bass program (same SPMD program for all 8 cores)."""
    nc = bacc.Bacc("TRN2", target_bir_lowering=False, debug=False)
    f32, bf16 = mybir.dt.float32, mybir.dt.bfloat16

    # ---- DRAM I/O (per-core tensors; see _prep for layouts) ----
    xT_d = nc.dram_tensor("xT", [8, 128, B], bf16, kind="ExternalInput").ap()
    w_in_d = nc.dram_tensor("w_in", [8, 128, OC], bf16, kind="ExternalInput").ap()
    s1_d = nc.dram_tensor("s1", [NKT, 128, OC], bf16, kind="ExternalInput").ap()
    s2_d = nc.dram_tensor("s2", [NKT, 128, OC], bf16, kind="ExternalInput").ap()
    w_out_d = nc.dram_tensor("w_out", [8, 128, OPAD], bf16, kind="ExternalInput").ap()
    b_in_d = nc.dram_tensor("b_in", [128, 8], f32, kind="ExternalInput").ap()
    b1_d = nc.dram_tensor("b1", [128, 8], f32, kind="ExternalInput").ap()
    b2_d = nc.dram_tensor("b2", [128, 8], f32, kind="ExternalInput").ap()
    out_d = nc.dram_tensor("out", [BC, OPAD], f32, kind="ExternalOutput").ap()

    # ---- internal DRAM (collective bounce buffers) ----
    h1b = nc.dram_tensor("h1b", [OC, B], bf16).ap()
    h2b = nc.dram_tensor("h2b", [OC, B], bf16).ap()
    h1g = nc.dram_tensor("h1g", [NUM_MID, B], bf16, addr_space="Shared").ap()
    h2g = nc.dram_tensor("h2g", [NUM_MID, B], bf16, addr_space="Shared").ap()
    outp = nc.dram_tensor("outp", [B, OPAD], f32).ap()
    outr = nc.dram_tensor("outr", [BC, OPAD], f32).ap()

    with tile.TileContext(nc) as tc:
        with (
            tc.tile_pool(name="const", bufs=1) as cpool,
            tc.tile_pool(name="hfull", bufs=1) as hpool,
            tc.tile_pool(name="wstream", bufs=4) as wpool,
            tc.tile_pool(name="hchunk", bufs=2) as hcpool,
            tc.tile_pool(name="ps", bufs=8, space="PSUM") as pp,
        ):
            # biases (tiny, f32): column ot is the bias for o-tile ot
            b_in = cpool.tile([128, 8], f32)
            b1 = cpool.tile([128, 8], f32)
            b2 = cpool.tile([128, 8], f32)
            nc.sync.dma_start(out=b_in[:], in_=b_in_d)
            nc.sync.dma_start(out=b1[:], in_=b1_d)
            nc.sync.dma_start(out=b2[:], in_=b2_d)

            # full x.T (k-major), replicated: [128, 8kt, 512b]
            xT = cpool.tile([128, 8, B], bf16)
            nc.sync.dma_start(out=xT[:], in_=xT_d.rearrange("kt p b -> p kt b"))

            def mk_psums(tag):
                return [pp.tile([128, B], f32, tag="ps", name=f"ps_{tag}{i}")
                        for i in range(8)]

            def close_layer(psums, bias, hc):
                """ReLU+bias each psum bank -> bf16 k-major chunk [128, 8, B]."""
                for ot in range(8):
                    nc.scalar.activation(
                        hc[:, ot, :], psums[ot][:],
                        mybir.ActivationFunctionType.Relu,
                        bias=bias[:, ot:ot + 1],
                    )

            # ---------------- in-layer: h1c = relu(x @ W_in.T + b_in) -------
            ps = mk_psums("a")
            for kt in range(8):
                wt = wpool.tile([128, OC], bf16, tag="w")
                nc.sync.dma_start(out=wt[:], in_=w_in_d[kt])
                for ot in range(8):
                    nc.tensor.matmul(
                        ps[ot][:], wt[:, ot * 128:(ot + 1) * 128], xT[:, kt, :],
                        start=(kt == 0), stop=(kt == 7),
                    )
            h1c = hcpool.tile([128, 8, B], bf16, tag="hc")
            close_layer(ps, b_in, h1c)
            for ot in range(8):
                nc.sync.dma_start(out=h1b[ot * 128:(ot + 1) * 128, :],
                                  in_=h1c[:, ot, :])
            nc.gpsimd.collective_compute(
                "AllGather", mybir.AluOpType.bypass, replica_groups=RG,
                ins=[h1b], outs=[h1g],
            )

            # ---------------- condensed layers ------------------------------
            def cond_layer(tag, hg_dram, s_dram, bias, prev_local=None):
                """h_next_chunk = relu(h_prev @ S_chunk + b_chunk).

                hg_dram: AllGathered k-major activations [8192, 512] in DRAM.
                Returns the SBUF chunk tile [128, 8, B].
                """
                # ACT-ring DMAs: don't block the S-stream on the sync ring
                hfull = hpool.tile([128, NKT, B], bf16, tag=f"hf{tag}")
                for kt in range(NKT):
                    nc.scalar.dma_start(out=hfull[:, kt, :],
                                        in_=hg_dram[kt * 128:(kt + 1) * 128, :])
                ps = mk_psums(tag)
                for kt in range(NKT):
                    st = wpool.tile([128, OC], bf16, tag="w")
                    nc.sync.dma_start(out=st[:], in_=s_dram[kt])
                    for ot in range(8):
                        nc.tensor.matmul(
                            ps[ot][:], st[:, ot * 128:(ot + 1) * 128],
                            hfull[:, kt, :],
                            start=(kt == 0), stop=(kt == NKT - 1),
                        )
                hc = hcpool.tile([128, 8, B], bf16, tag="hc")
                close_layer(ps, bias, hc)
                return hc

            h2c = cond_layer("b", h1g, s1_d, b1)
            for ot in range(8):
                nc.sync.dma_start(out=h2b[ot * 128:(ot + 1) * 128, :],
                                  in_=h2c[:, ot, :])
            nc.gpsimd.collective_compute(
                "AllGather", mybir.AluOpType.bypass, replica_groups=RG,
                ins=[h2b], outs=[h2g],
            )

            h3c = cond_layer("c", h2g, s2_d, b2)

            # ---------------- out-layer (k-sharded) + ReduceScatter ---------
            # partial[b, o] = h3_chunk @ W_out_chunk ; RS(add) over cores
            pso = mk_psums("o")
            for kt in range(8):
                wt = wpool.tile([128, OPAD], bf16, tag="w")
                nc.sync.dma_start(out=wt[:], in_=w_out_d[kt])
                for bc in range(4):
                    for oh in range(2):
                        nc.tensor.matmul(
                            pso[bc * 2 + oh][:],
                            h3c[:, kt, bc * 128:(bc + 1) * 128],
                            wt[:, oh * 512:(oh + 1) * 512],
                            start=(kt == 0), stop=(kt == 7),
                        )
            for bc in range(4):
                for oh in range(2):
                    osb = hcpool.tile([128, 512], f32, tag="osb")
                    nc.vector.tensor_copy(osb[:], pso[bc * 2 + oh][:])
                    nc.sync.dma_start(
                        out=outp[bc * 128:(bc + 1) * 128,
                                 oh * 512:(oh + 1) * 512],
                        in_=osb[:])
            nc.gpsimd.collective_compute(
                "ReduceScatter", mybir.AluOpType.add, replica_groups=RG,
                ins=[outp], outs=[outr],
            )
            nc.sync.dma_start(out=out_d, in_=outr)

    nc.compile()
    return nc


def _build_S(Wm, idx):
    S = np.zeros((NUM_MID, NUM_MID), np.float32)
    cols = np.repeat(np.arange(NUM_MID), FAN_IN)
    np.add.at(S, (idx.reshape(-1), cols), Wm.reshape(-1).astype(np.float32))
    return S


def _prep(x, W_in, b_in, W_mid, b_mid, W_out, b_out, indx_seqs):
    """Host-side compile-time transforms of weights; per-core input maps."""
    idx = np.asarray(indx_seqs).astype(np.int64)

    xT = np.ascontiguousarray(
        np.asarray(x, np.float32).T.reshape(8, 128, B).astype(BF16))

    W_inT = np.asarray(W_in, np.float32).T          # [1024, 8192]
    S1 = _build_S(np.asarray(W_mid[0]), idx)        # [8192, 8192]
    S2 = _build_S(np.asarray(W_mid[1]), idx)
    W_outT = np.zeros((NUM_MID, OPAD), np.float32)  # [8192, 1024]
    W_outT[:, :NUM_OUT] = np.asarray(W_out, np.float32).T

    def bias_tiles(b, c):
        return np.ascontiguousarray(
            np.asarray(b, np.float32)[c * OC:(c + 1) * OC].reshape(8, 128).T)

    in_maps = []
    for c in range(NCORES):
        sl = slice(c * OC, (c + 1) * OC)
        m = {
            "xT": xT,
            "w_in": np.ascontiguousarray(
                W_inT[:, sl].reshape(8, 128, OC).astype(BF16)),
            "s1": np.ascontiguousarray(
                S1[:, sl].reshape(NKT, 128, OC).astype(BF16)),
            "s2": np.ascontiguousarray(
                S2[:, sl].reshape(NKT, 128, OC).astype(BF16)),
            "w_out": np.ascontiguousarray(
                W_outT[sl, :].reshape(8, 128, OPAD).astype(BF16)),
            "b_in": bias_tiles(b_in, c),
            "b1": bias_tiles(b_mid[0], c),
            "b2": bias_tiles(b_mid[1], c),
        }
        in_maps.append(m)
    return in_maps, np.asarray(b_out, np.float32)


def kernel(x, W_in, b_in, W_mid, b_mid, W_out, b_out, indx_seqs):
    global LAST_RESULT
    if "nc" not in _cache:
        _cache["nc"] = _build_nc()
    nc = _cache["nc"]

    in_maps, b_out_f = _prep(x, W_in, b_in, W_mid, b_mid, W_out, b_out,
                             indx_seqs)

    res = bass_utils.run_bass_kernel_spmd(
        nc, in_maps, core_ids=list(range(NCORES)),
        trace=bool(int(os.environ.get("KERNEL_TRACE", "0"))),
    )
    LAST_RESULT = res

    out = np.concatenate([r["out"][:, :NUM_OUT] for r in res.results], axis=0)
    return (out + b_out_f[None, :]).astype(np.float32)
